# revision 51
# baseline (speedup 1.0000x reference)
"""LMMD (DSAN local MMD) loss on 8 Trainium2 NeuronCores — cyclic-support V5.

Math (reference):
    X = concat(source, target)                    # [N=4096, D=1024]
    l2[i,j] = max(|x_i|^2 + |x_j|^2 - 2 x_i.x_j, 0)
    bw      = sum(l2) / (N^2 - N) / 4
    K       = sum_q exp(-l2 / (bw * 2^q)),  q = 0..4
    loss    = sum_c v_c^T K v_c / 12,  V = [s_norm; -t_norm]  (rank-12 weights)

V5 design:
  * Cyclic 16-tile support: core c holds X columns for tiles
    (4c + S0) mod 32 with S0 = {0..7, 12..19}.  The 528 unordered
    128-tile pairs partition into 8 identical 68-job lists (60 weight-2
    oriented pairs covering every (difference, residue) cell once, 4
    weight-1 distance-16 jobs computed twice with opposite orientations,
    4 weight-1 diagonals), so every core runs the SAME program on a
    rotated tile set and per-core X DMA halves to 16 KB/partition.
  * Jobs stream through 9 batches (6|8x7|6 jobs).  Per batch: fp8
    DoubleRow gram into one 2-bank PSUM tile, three bias-free ACT exps
    (e4 = exp(2c4 G) full width, e1/e0 heads straight from the gram at
    8x/16x scale), DVE squaring e3/e2/e1-tail, Pool squares the e0 tail
    from DVE's e1 region only (fully decoupled engine chains).  The
    j-side RBF factor exp(-c_q sq_j) is folded into per-q bf16 vt
    tables; the i-side factor is applied on the host.
  * Weighted reduce keeps es stationary (12-wide moving vt), accumulating
    R_q[i, cls] into two PSUM tiles (q>=2 / q<=1) so the high-q drain
    overlaps the low-q matmuls.  PSUM has_written semantics: one
    start per bank, first-touch overwrites via cleared bits.
"""

import numpy as np
import ml_dtypes

import concourse.bass as bass
from concourse import bacc
import concourse.mybir as mybir
import concourse.tile as tile
from concourse.bass_utils import run_bass_kernel_spmd

B = 2048
D = 1024
C = 12
NCORES = 8
N = 2 * B                 # 4096 total samples
NT = N // 128             # 32 j-tiles
NKC = D // 128            # 8 contraction chunks
NKP = NKC // 2            # 4 DoubleRow chunk-pairs
NQ = 5
M = 16                    # tiles in the cyclic support
NI = 8                    # i-side slots (positions 0..7)
WLAG = 3                  # batches of lag between es production and weighted

S0 = list(range(0, 8)) + list(range(12, 20))

# btab layout (bf16): vt2 [5*M*C] | vt1 [5*8*C] | scales [8]
# scales: [2c4, 2c3, 2c2, 2c1, 2c0, 0(bias), 0, 0]
VT2_COLS = NQ * M * C
VT1_COLS = NQ * 8 * C
NSCL = 8
BT_COLS = VT2_COLS + VT1_COLS + NSCL

F8NP = ml_dtypes.float8_e4m3
BFNP = ml_dtypes.bfloat16

_BUILT = None


def _plan_jobs():
    """Deterministic job plan: 68 (jpos, ipos, weight, vt1slot) tuples in
    S0-local positions, every global pair covered exactly once."""
    import itertools

    Sset = set(S0)
    pos = {t: i for i, t in enumerate(S0)}
    ILOCAL = set(range(0, 8))
    pairs = []
    for a, b in itertools.combinations(S0, 2):
        if a not in ILOCAL and b not in ILOCAL:
            continue
        d = (b - a) % 32
        cells = set()
        for (base, dd) in ((a, d), (b, (32 - d) % 32)):
            if 1 <= dd <= 15:
                cells.add((dd, base % 4))
        if cells:
            pairs.append(((a, b), sorted(cells)))
    cells_needed = [(d, r) for d in range(1, 16) for r in range(4)]
    cell_idx = {c: i for i, c in enumerate(cells_needed)}
    adj = [[] for _ in cells_needed]
    for pi, (fs, cells) in enumerate(pairs):
        for cc in cells:
            if cc in cell_idx:
                adj[cell_idx[cc]].append(pi)
    for ci in range(len(adj)):
        adj[ci].sort(key=lambda pi: max(pairs[pi][0]))
    match_pair = {}
    match_cell = [None] * len(cells_needed)

    def aug(ci, seen):
        for pi in adj[ci]:
            if pi in seen:
                continue
            seen.add(pi)
            if pi not in match_pair or aug(match_pair[pi], seen):
                match_pair[pi] = ci
                match_cell[ci] = pi
                return True
        return False

    for ci in range(len(cells_needed)):
        assert aug(ci, set())
    jobs = []
    for ci, pi in enumerate(match_cell):
        (a, b) = pairs[pi][0]
        i_t = a if a in ILOCAL else b
        j_t = b if i_t == a else a
        jobs.append((pos[j_t], pos[i_t], 2, -1))
    for x in range(4):                       # d16, weight 1, computed twice
        jobs.append((pos[x + 16], pos[x], 1, x))
    for x in range(4):                       # diagonal, weight 1
        jobs.append((pos[x], pos[x], 1, 4 + x))
    # order by data arrival (4-position DMA chunks), then j for locality
    jobs.sort(key=lambda jb: (max(jb[0] // 4, jb[1] // 4), jb[0], jb[1]))
    return jobs


JOBS = _plan_jobs()
BATCH_SIZES = [10, 12, 12, 12, 12, 6, 4]
assert sum(BATCH_SIZES) == len(JOBS) == 68
NPOOL_FREE = 2            # trailing batches whose e0 tail runs on DVE, not Pool
BMAX = max(BATCH_SIZES) * 128
BATCHES = []
_k = 0
for bs in BATCH_SIZES:
    BATCHES.append(JOBS[_k : _k + bs])
    _k += bs
NB = len(BATCHES)


def _asplit(w):
    # ACT's exp(8sG)/exp(16sG) head widths (e1/e0); Pool squares e0[a:]
    if w <= 768:
        return 128
    return 288 if w <= 1280 else 352


def _build_program():
    fp32 = mybir.dt.float32
    bf16 = mybir.dt.bfloat16
    f8 = mybir.dt.float8e4
    Exp = mybir.ActivationFunctionType.Exp
    Copy = mybir.ActivationFunctionType.Copy
    DR = mybir.MatmulPerfMode.DoubleRow

    nc = bacc.Bacc()
    # host-pretransposed: xtb[p, k, t*128+j] = X[(4c+S0[t])*128+j, k*128+p]
    xtb = nc.declare_dram_parameter("xtb", [128, NKC, M * 128], f8, isOutput=False)
    btab = nc.declare_dram_parameter("btab", [128, BT_COLS], bf16, isOutput=False)
    rout = nc.declare_dram_parameter("r_out", [128, NQ * NI * C], fp32, isOutput=True)

    with tile.TileContext(nc) as tc:
        with (
            tc.tile_pool(name="singles", bufs=1) as singles,
            tc.tile_pool(name="epool", bufs=5) as epool,
            tc.tile_pool(name="gpsum", bufs=2, space="PSUM") as gpsum,
            tc.tile_pool(name="rqpsum", bufs=1, space="PSUM") as rqpsum,
        ):
            xtb_sb = singles.tile([128, NKC, M * 128], f8)
            btab_sb = singles.tile([128, BT_COLS], bf16)
            # PE p-state warm-up: ~3us of dummy matmuls on a never-written
            # scratch tile so the first real gram runs at full clock.  The
            # results land in a recycled gpsum generation nobody reads.
            wsrc = singles.tile([128, 2, 512], f8)
            nc.gpsimd.memset(wsrc, 0.0)
            wu = gpsum.tile([128, BMAX], fp32, tag="g", name="gwarm")
            for k in range(14):
                nc.tensor.matmul(
                    wu[:, 0:512],
                    lhsT=wsrc[:, :, 0:128],
                    rhs=wsrc,
                    start=(k == 0),
                    stop=(k == 13),
                    perf_mode=DR,
                )
            # DMA stream: first batch's tiles (positions 0-3) in two k-halves
            # so gram m=0,1 starts early; scales early (tiny); remaining tile
            # chunks; the bulk vt table after the second chunk.
            nc.sync.dma_start(out=xtb_sb[:, 0:4, 0:512], in_=xtb[:, 0:4, 0:512])
            nc.sync.dma_start(out=xtb_sb[:, 4:8, 0:512], in_=xtb[:, 4:8, 0:512])
            nc.sync.dma_start(
                out=btab_sb[:, VT2_COLS + VT1_COLS :],
                in_=btab[:, VT2_COLS + VT1_COLS :],
            )
            scl_s = singles.tile([128, NSCL], fp32)
            nc.vector.tensor_copy(scl_s, btab_sb[:, VT2_COLS + VT1_COLS :])
            # Exp-table warm-up reads a const tile so it runs during the DMA
            # head instead of waiting for the scale fetch.
            warm_in = singles.tile([128, 4], fp32)
            nc.gpsimd.memset(warm_in, 0.0)
            warm = singles.tile([128, 4], fp32)
            nc.scalar.activation(warm, warm_in, Exp)
            nc.sync.dma_start(out=xtb_sb[:, :, 512:1024], in_=xtb[:, :, 512:1024])
            nc.sync.dma_start(
                out=btab_sb[:, 0 : VT2_COLS + VT1_COLS],
                in_=btab[:, 0 : VT2_COLS + VT1_COLS],
            )
            nc.sync.dma_start(out=xtb_sb[:, :, 1024:1536], in_=xtb[:, :, 1024:1536])
            nc.sync.dma_start(out=xtb_sb[:, :, 1536:2048], in_=xtb[:, :, 1536:2048])

            # R accumulators: hi = q {4,3,2}, lo = q {1,0}; one bank each
            rq_hi = rqpsum.tile([128, 3 * NI * C], fp32, tag="rqh", name="rq_hi")
            rq_lo = rqpsum.tile([128, 2 * NI * C], fp32, tag="rql", name="rq_lo")

            def rq_slice(q, islot):
                if q >= 2:
                    base = ((q - 2) * NI + islot) * C
                    return rq_hi[:, base : base + C]
                base = (q * NI + islot) * C
                return rq_lo[:, base : base + C]

            first_mm = {"hi": True, "lo": True}
            n_emitted = [0]

            def emit_weighted(bi, es):
                jobs = BATCHES[bi]
                for q in range(NQ - 1, -1, -1):
                    for jj, (jpos, ipos, w, vt1slot) in enumerate(jobs):
                        if w == 2:
                            vb = (q * M + jpos) * C
                        else:
                            vb = VT2_COLS + (q * 8 + vt1slot) * C
                        key = "hi" if q >= 2 else "lo"
                        n_emitted[0] += 1
                        last = n_emitted[0] == NB and False
                        nc.tensor.matmul(
                            rq_slice(q, ipos),
                            lhsT=es[q][:, jj * 128 : (jj + 1) * 128],
                            rhs=btab_sb[:, vb : vb + C],
                            start=first_mm[key],
                            stop=(bi == NB - 1 and q in (2, 0) and jj == len(jobs) - 1),
                        )
                        first_mm[key] = False

            pending = []
            for bi, jobs in enumerate(BATCHES):
                w = len(jobs) * 128
                a = _asplit(w)
                gt = gpsum.tile([128, BMAX], fp32, tag="g", name=f"g{bi}")
                for jj, (jpos, ipos, _, _) in enumerate(jobs):
                    for m in range(NKP):
                        nc.tensor.matmul(
                            gt[:, jj * 128 : (jj + 1) * 128],
                            lhsT=xtb_sb[:, 2 * m : 2 * m + 2, jpos * 128 : (jpos + 1) * 128],
                            rhs=xtb_sb[:, 2 * m : 2 * m + 2, ipos * 128 : (ipos + 1) * 128],
                            start=(m == 0),
                            stop=(m == NKP - 1),
                            perf_mode=DR,
                        )
                es = {q: epool.tile([128, BMAX], bf16, tag=f"e{q}", name=f"e{q}b{bi}") for q in range(NQ)}
                zb = scl_s[:, 5:6]
                nc.scalar.activation(
                    es[4][:, 0:w], gt[:, 0:w], Exp, bias=zb, scale=scl_s[:, 0:1],
                )
                nc.scalar.activation(
                    es[1][:, 0:a], gt[:, 0:a], Exp, bias=zb, scale=scl_s[:, 3:4],
                )
                nc.scalar.activation(
                    es[0][:, 0:a], gt[:, 0:a], Exp, bias=zb, scale=scl_s[:, 4:5],
                )
                nc.vector.tensor_mul(es[3][:, 0:w], es[4][:, 0:w], es[4][:, 0:w])
                nc.vector.tensor_mul(es[2][:, 0:w], es[3][:, 0:w], es[3][:, 0:w])
                if a < w:
                    nc.vector.tensor_mul(es[1][:, a:w], es[2][:, a:w], es[2][:, a:w])
                    if bi >= NB - NPOOL_FREE:
                        nc.vector.tensor_mul(es[0][:, a:w], es[1][:, a:w], es[1][:, a:w])
                    else:
                        nc.gpsimd.tensor_mul(es[0][:, a:w], es[1][:, a:w], es[1][:, a:w])
                pending.append((bi, es))
                if len(pending) > WLAG:
                    emit_weighted(*pending.pop(0))
            for item in pending:
                emit_weighted(*item)

            # tail: hi drains on DVE while the low-q matmuls still run; lo on
            # the by-then-idle ACT; one bf16 DMA.
            stg = singles.tile([128, NQ * NI * C], fp32)
            nc.vector.tensor_copy(stg[:, 2 * NI * C :], rq_hi)
            nc.scalar.activation(stg[:, 0 : 2 * NI * C], rq_lo, Copy)
            nc.sync.dma_start(out=rout[:], in_=stg)

    nc.compile()
    return nc


def _prep(source, target, source_label, target_logits):
    X = np.concatenate([np.asarray(source), np.asarray(target)], axis=0)
    X64 = X.astype(np.float64)
    sq = np.einsum("nd,nd->n", X64, X64)
    colsum = X64.sum(axis=0)
    sum_l2 = 2.0 * N * sq.sum() - 2.0 * (colsum @ colsum)
    bw = sum_l2 / (N * N - N) / (2.0 ** (NQ // 2))
    cq = np.array([1.0 / (bw * 2.0**q) for q in range(NQ)])  # [5]

    sl = np.asarray(source_label, np.float64)
    tl = np.asarray(target_logits, np.float64)
    ssum = sl.sum(0)
    s_norm = np.where(ssum > 0, sl / np.where(ssum > 0, ssum, 1.0), 0.0)
    tsum = tl.sum(0)
    t_norm = np.where(tsum > 0, tl / np.where(tsum > 0, tsum, 1.0), 0.0)
    s_pres = np.zeros(C)
    np.add.at(s_pres, sl.argmax(1), 1.0)
    t_pres = np.zeros(C)
    np.add.at(t_pres, tl.argmax(1), 1.0)
    common = ((s_pres > 0) & (t_pres > 0)).astype(np.float64)
    V = np.concatenate([s_norm * common, -t_norm * common], axis=0)  # [N, C]

    fq = np.exp(-np.outer(cq, sq))                        # [5, N]

    X8 = X.astype(F8NP)                                   # [N, D]
    xt8 = np.ascontiguousarray(
        X8.T.reshape(NKC, 128, N).transpose(1, 0, 2)      # [128, 8, N]
    )
    return X, sq, cq, V, fq, xt8


def _core_inputs(c, cq, V, fq, xt8):
    gtiles = [(4 * c + s) % 32 for s in S0]
    xtb = np.ascontiguousarray(
        xt8.reshape(128, NKC, NT, 128)[:, :, gtiles, :].reshape(128, NKC, M * 128)
    )
    # vt2[q, t] = 2 V f_q at global tile gtiles[t]
    Vt = V.reshape(NT, 128, C)[gtiles]                    # [M, 128, C]
    fqt = fq.reshape(NQ, NT, 128)[:, gtiles]              # [NQ, M, 128]
    Vq = Vt[None] * fqt[..., None]                        # [NQ, M, 128, C]
    vt2 = (2.0 * Vq).transpose(2, 0, 1, 3).reshape(128, NQ * M * C)
    # vt1 slots: 0..3 = d16 jobs (j = position 8+... tile 16+x), 4..7 = diag x
    vt1 = np.zeros((128, NQ, 8, C))
    for x in range(4):
        jpos = S0.index(x + 16)
        vt1[:, :, x, :] = Vq[:, jpos].transpose(1, 0, 2)
        vt1[:, :, 4 + x, :] = Vq[:, S0.index(x)].transpose(1, 0, 2)
    vt1 = vt1.reshape(128, NQ * 8 * C)
    scl = np.zeros((128, 8))
    for k in range(NQ):
        scl[:, k] = 2.0 * cq[4 - k]
    btab = np.ascontiguousarray(
        np.concatenate([vt2, vt1, scl], axis=1)
    ).astype(BFNP)
    return {"xtb": xtb, "btab": btab}


def _postprocess(results, sq, cq, V):
    # loss = 1/12 sum_q sum_i alpha_q[i] * (sum_cls V[i,cls] R_q[i,cls])
    loss = 0.0
    for c in range(NCORES):
        r = np.asarray(results[c]["r_out"], np.float64).reshape(128, NQ, NI, C)
        for s in range(NI):
            gt_ = (4 * c + S0[s]) % 32
            gi = gt_ * 128 + np.arange(128)
            alpha = np.exp(-np.outer(cq, sq[gi]))         # [NQ, 128]
            loss += np.einsum("qp,pc,pqc->", alpha, V[gi], r[:, :, s, :])
    return loss / C


def _run(in_maps, trace=False, **kw):
    global _BUILT
    if _BUILT is None:
        _BUILT = _build_program()
    return run_bass_kernel_spmd(_BUILT, in_maps, list(range(NCORES)), trace=trace, **kw)


def kernel(source, target, source_label, target_logits, _trace=False, _ret_bkr=False):
    X, sq, cq, V, fq, xt8 = _prep(source, target, source_label, target_logits)
    in_maps = [_core_inputs(c, cq, V, fq, xt8) for c in range(NCORES)]
    bkr = None
    for attempt in range(3):
        try:
            bkr = _run(in_maps, trace=_trace)
            break
        except Exception:
            if attempt == 2:
                raise
            import time as _time

            _time.sleep(2.0)
    loss = _postprocess(bkr.results, sq, cq, V)
    out = np.float32(loss)
    if _ret_bkr:
        return out, bkr
    return out


# revision 55
# speedup vs baseline: 1.0051x; 1.0051x over previous
"""LMMD (DSAN local MMD) loss on 8 Trainium2 NeuronCores — cyclic-support V5.

Math (reference):
    X = concat(source, target)                    # [N=4096, D=1024]
    l2[i,j] = max(|x_i|^2 + |x_j|^2 - 2 x_i.x_j, 0)
    bw      = sum(l2) / (N^2 - N) / 4
    K       = sum_q exp(-l2 / (bw * 2^q)),  q = 0..4
    loss    = sum_c v_c^T K v_c / 12,  V = [s_norm; -t_norm]  (rank-12 weights)

V5 design:
  * Cyclic 16-tile support: core c holds X columns for tiles
    (4c + S0) mod 32 with S0 = {0..7, 12..19}.  The 528 unordered
    128-tile pairs partition into 8 identical 68-job lists (60 weight-2
    oriented pairs covering every (difference, residue) cell once, 4
    weight-1 distance-16 jobs computed twice with opposite orientations,
    4 weight-1 diagonals), so every core runs the SAME program on a
    rotated tile set and per-core X DMA halves to 16 KB/partition.
  * Jobs stream through 9 batches (6|8x7|6 jobs).  Per batch: fp8
    DoubleRow gram into one 2-bank PSUM tile, three bias-free ACT exps
    (e4 = exp(2c4 G) full width, e1/e0 heads straight from the gram at
    8x/16x scale), DVE squaring e3/e2/e1-tail, Pool squares the e0 tail
    from DVE's e1 region only (fully decoupled engine chains).  The
    j-side RBF factor exp(-c_q sq_j) is folded into per-q bf16 vt
    tables; the i-side factor is applied on the host.
  * Weighted reduce keeps es stationary (12-wide moving vt), accumulating
    R_q[i, cls] into two PSUM tiles (q>=2 / q<=1) so the high-q drain
    overlaps the low-q matmuls.  PSUM has_written semantics: one
    start per bank, first-touch overwrites via cleared bits.
"""

import numpy as np
import ml_dtypes

import concourse.bass as bass
from concourse import bacc
import concourse.mybir as mybir
import concourse.tile as tile
from concourse.bass_utils import run_bass_kernel_spmd

B = 2048
D = 1024
C = 12
NCORES = 8
N = 2 * B                 # 4096 total samples
NT = N // 128             # 32 j-tiles
NKC = D // 128            # 8 contraction chunks
NKP = NKC // 2            # 4 DoubleRow chunk-pairs
NQ = 5
M = 16                    # tiles in the cyclic support
NI = 8                    # i-side slots (positions 0..7)
WLAG = 3                  # batches of lag between es production and weighted

S0 = list(range(0, 8)) + list(range(12, 20))

# btab layout (bf16): vt2 [5*M*C] | vt1 [5*8*C] | scales [8]
# scales: [2c4, 2c3, 2c2, 2c1, 2c0, 0(bias), 0, 0]
VT2_COLS = NQ * M * C
VT1_COLS = NQ * 8 * C
NSCL = 8
BT_COLS = VT2_COLS + VT1_COLS + NSCL

F8NP = ml_dtypes.float8_e4m3
BFNP = ml_dtypes.bfloat16

_BUILT = None


def _plan_jobs():
    """Deterministic job plan: 68 (jpos, ipos, weight, vt1slot) tuples in
    S0-local positions, every global pair covered exactly once."""
    import itertools

    Sset = set(S0)
    pos = {t: i for i, t in enumerate(S0)}
    ILOCAL = set(range(0, 8))
    pairs = []
    for a, b in itertools.combinations(S0, 2):
        if a not in ILOCAL and b not in ILOCAL:
            continue
        d = (b - a) % 32
        cells = set()
        for (base, dd) in ((a, d), (b, (32 - d) % 32)):
            if 1 <= dd <= 15:
                cells.add((dd, base % 4))
        if cells:
            pairs.append(((a, b), sorted(cells)))
    cells_needed = [(d, r) for d in range(1, 16) for r in range(4)]
    cell_idx = {c: i for i, c in enumerate(cells_needed)}
    adj = [[] for _ in cells_needed]
    for pi, (fs, cells) in enumerate(pairs):
        for cc in cells:
            if cc in cell_idx:
                adj[cell_idx[cc]].append(pi)
    for ci in range(len(adj)):
        adj[ci].sort(key=lambda pi: max(pairs[pi][0]))
    match_pair = {}
    match_cell = [None] * len(cells_needed)

    def aug(ci, seen):
        for pi in adj[ci]:
            if pi in seen:
                continue
            seen.add(pi)
            if pi not in match_pair or aug(match_pair[pi], seen):
                match_pair[pi] = ci
                match_cell[ci] = pi
                return True
        return False

    for ci in range(len(cells_needed)):
        assert aug(ci, set())
    jobs = []
    for ci, pi in enumerate(match_cell):
        (a, b) = pairs[pi][0]
        i_t = a if a in ILOCAL else b
        j_t = b if i_t == a else a
        jobs.append((pos[j_t], pos[i_t], 2, -1))
    for x in range(4):                       # d16, weight 1, computed twice
        jobs.append((pos[x + 16], pos[x], 1, x))
    for x in range(4):                       # diagonal, weight 1
        jobs.append((pos[x], pos[x], 1, 4 + x))
    # order by data arrival (4-position DMA chunks), then j for locality
    jobs.sort(key=lambda jb: (max(jb[0] // 4, jb[1] // 4), jb[0], jb[1]))
    return jobs


JOBS = _plan_jobs()
BATCH_SIZES = [10, 12, 12, 12, 12, 6, 4]
assert sum(BATCH_SIZES) == len(JOBS) == 68
NPOOL_FREE = 2            # trailing batches whose e0 tail runs on DVE, not Pool
BMAX = max(BATCH_SIZES) * 128
BATCHES = []
_k = 0
for bs in BATCH_SIZES:
    BATCHES.append(JOBS[_k : _k + bs])
    _k += bs
NB = len(BATCHES)


def _asplit(w):
    # ACT's exp(8sG)/exp(16sG) head widths (e1/e0); Pool squares e0[a:]
    if w <= 768:
        return 128
    return 288 if w <= 1280 else 352


def _build_program():
    fp32 = mybir.dt.float32
    bf16 = mybir.dt.bfloat16
    f8 = mybir.dt.float8e4
    Exp = mybir.ActivationFunctionType.Exp
    Copy = mybir.ActivationFunctionType.Copy
    DR = mybir.MatmulPerfMode.DoubleRow

    nc = bacc.Bacc()
    # host-pretransposed: xtb[p, k, t*128+j] = X[(4c+S0[t])*128+j, k*128+p]
    xtb = nc.declare_dram_parameter("xtb", [128, NKC, M * 128], f8, isOutput=False)
    btab = nc.declare_dram_parameter("btab", [128, BT_COLS], bf16, isOutput=False)
    rout = nc.declare_dram_parameter("r_out", [128, NQ * NI * C], fp32, isOutput=True)

    with tile.TileContext(nc) as tc:
        with (
            tc.tile_pool(name="singles", bufs=1) as singles,
            tc.tile_pool(name="epool", bufs=6) as epool,
            tc.tile_pool(name="gpsum", bufs=2, space="PSUM") as gpsum,
            tc.tile_pool(name="rqpsum", bufs=1, space="PSUM") as rqpsum,
        ):
            xtb_sb = singles.tile([128, NKC, M * 128], f8)
            btab_sb = singles.tile([128, BT_COLS], bf16)
            # PE p-state warm-up: ~3us of dummy matmuls on a never-written
            # scratch tile so the first real gram runs at full clock.  The
            # results land in a recycled gpsum generation nobody reads.
            wsrc = singles.tile([128, 2, 512], f8)
            nc.gpsimd.memset(wsrc, 0.0)
            wu = gpsum.tile([128, BMAX], fp32, tag="g", name="gwarm")
            for k in range(14):
                nc.tensor.matmul(
                    wu[:, 0:512],
                    lhsT=wsrc[:, :, 0:128],
                    rhs=wsrc,
                    start=(k == 0),
                    stop=(k == 13),
                    perf_mode=DR,
                )
            # DMA stream: first batch's tiles (positions 0-3) in two k-halves
            # so gram m=0,1 starts early; scales early (tiny); remaining tile
            # chunks; the bulk vt table after the second chunk.
            nc.sync.dma_start(out=xtb_sb[:, 0:4, 0:512], in_=xtb[:, 0:4, 0:512])
            nc.sync.dma_start(out=xtb_sb[:, 4:8, 0:512], in_=xtb[:, 4:8, 0:512])
            nc.sync.dma_start(
                out=btab_sb[:, VT2_COLS + VT1_COLS :],
                in_=btab[:, VT2_COLS + VT1_COLS :],
            )
            scl_s = singles.tile([128, NSCL], fp32)
            nc.vector.tensor_copy(scl_s, btab_sb[:, VT2_COLS + VT1_COLS :])
            # Exp-table warm-up reads a const tile so it runs during the DMA
            # head instead of waiting for the scale fetch.
            warm_in = singles.tile([128, 4], fp32)
            nc.gpsimd.memset(warm_in, 0.0)
            warm = singles.tile([128, 4], fp32)
            nc.scalar.activation(warm, warm_in, Exp)
            nc.sync.dma_start(out=xtb_sb[:, :, 512:1024], in_=xtb[:, :, 512:1024])
            nc.sync.dma_start(
                out=btab_sb[:, 0 : VT2_COLS + VT1_COLS],
                in_=btab[:, 0 : VT2_COLS + VT1_COLS],
            )
            nc.sync.dma_start(out=xtb_sb[:, :, 1024:1536], in_=xtb[:, :, 1024:1536])
            nc.sync.dma_start(out=xtb_sb[:, :, 1536:2048], in_=xtb[:, :, 1536:2048])

            # R accumulators: hi = q {4,3,2}, lo = q {1,0}; one bank each
            rq_hi = rqpsum.tile([128, 3 * NI * C], fp32, tag="rqh", name="rq_hi")
            rq_lo = rqpsum.tile([128, 2 * NI * C], fp32, tag="rql", name="rq_lo")

            def rq_slice(q, islot):
                if q >= 2:
                    base = ((q - 2) * NI + islot) * C
                    return rq_hi[:, base : base + C]
                base = (q * NI + islot) * C
                return rq_lo[:, base : base + C]

            first_mm = {"hi": True, "lo": True}

            def emit_one(bi, es, q, jj, stop=False):
                jpos, ipos, w, vt1slot = BATCHES[bi][jj]
                if w == 2:
                    vb = (q * M + jpos) * C
                else:
                    vb = VT2_COLS + (q * 8 + vt1slot) * C
                key = "hi" if q >= 2 else "lo"
                nc.tensor.matmul(
                    rq_slice(q, ipos),
                    lhsT=es[q][:, jj * 128 : (jj + 1) * 128],
                    rhs=btab_sb[:, vb : vb + C],
                    start=first_mm[key],
                    stop=stop,
                )
                first_mm[key] = False

            def emit_main(bi, es):
                # q=4..1 plus the q0 jobs living in ACT's exp head — their es
                # is ready early.  q0 jobs overlapping Pool's slow e0 tail are
                # deferred a batch (emit_q0_tail) so they never park in PE's
                # 4-deep wait queue and block later grams.
                jobs = BATCHES[bi]
                a = _asplit(len(jobs) * 128)
                for q in range(NQ - 1, 0, -1):
                    for jj in range(len(jobs)):
                        emit_one(bi, es, q, jj,
                                 stop=(bi == NB - 1 and q == 2 and jj == len(jobs) - 1))
                for jj in range(len(jobs)):
                    if (jj + 1) * 128 <= a:
                        emit_one(bi, es, 0, jj)

            def emit_q0_tail(bi, es, stop=False):
                jobs = BATCHES[bi]
                a = _asplit(len(jobs) * 128)
                late = [jj for jj in range(len(jobs)) if (jj + 1) * 128 > a]
                for k, jj in enumerate(late):
                    emit_one(bi, es, 0, jj, stop=(stop and k == len(late) - 1))

            pending = []
            pending2 = []
            for bi, jobs in enumerate(BATCHES):
                w = len(jobs) * 128
                a = _asplit(w)
                gt = gpsum.tile([128, BMAX], fp32, tag="g", name=f"g{bi}")
                for jj, (jpos, ipos, _, _) in enumerate(jobs):
                    for m in range(NKP):
                        nc.tensor.matmul(
                            gt[:, jj * 128 : (jj + 1) * 128],
                            lhsT=xtb_sb[:, 2 * m : 2 * m + 2, jpos * 128 : (jpos + 1) * 128],
                            rhs=xtb_sb[:, 2 * m : 2 * m + 2, ipos * 128 : (ipos + 1) * 128],
                            start=(m == 0),
                            stop=(m == NKP - 1),
                            perf_mode=DR,
                        )
                es = {q: epool.tile([128, BMAX], bf16, tag=f"e{q}", name=f"e{q}b{bi}") for q in range(NQ)}
                zb = scl_s[:, 5:6]
                nc.scalar.activation(
                    es[4][:, 0:w], gt[:, 0:w], Exp, bias=zb, scale=scl_s[:, 0:1],
                )
                nc.scalar.activation(
                    es[1][:, 0:a], gt[:, 0:a], Exp, bias=zb, scale=scl_s[:, 3:4],
                )
                nc.scalar.activation(
                    es[0][:, 0:a], gt[:, 0:a], Exp, bias=zb, scale=scl_s[:, 4:5],
                )
                nc.vector.tensor_mul(es[3][:, 0:w], es[4][:, 0:w], es[4][:, 0:w])
                nc.vector.tensor_mul(es[2][:, 0:w], es[3][:, 0:w], es[3][:, 0:w])
                if a < w:
                    nc.vector.tensor_mul(es[1][:, a:w], es[2][:, a:w], es[2][:, a:w])
                    if bi >= NB - NPOOL_FREE:
                        nc.vector.tensor_mul(es[0][:, a:w], es[1][:, a:w], es[1][:, a:w])
                    else:
                        nc.gpsimd.tensor_mul(es[0][:, a:w], es[1][:, a:w], es[1][:, a:w])
                pending.append((bi, es))
                if len(pending) > WLAG:
                    item = pending.pop(0)
                    emit_main(*item)
                    pending2.append(item)
                if len(pending2) > 1:
                    emit_q0_tail(*pending2.pop(0))
            for item in pending:
                emit_main(*item)
                pending2.append(item)
            for k, item in enumerate(pending2):
                emit_q0_tail(*item, stop=(k == len(pending2) - 1))

            # tail: hi drains on DVE while the low-q matmuls still run; lo on
            # the by-then-idle ACT; one bf16 DMA.
            stg = singles.tile([128, NQ * NI * C], fp32)
            nc.vector.tensor_copy(stg[:, 2 * NI * C :], rq_hi)
            nc.scalar.activation(stg[:, 0 : 2 * NI * C], rq_lo, Copy)
            nc.sync.dma_start(out=rout[:], in_=stg)

    nc.compile()
    return nc


def _prep(source, target, source_label, target_logits):
    X = np.concatenate([np.asarray(source), np.asarray(target)], axis=0)
    X64 = X.astype(np.float64)
    sq = np.einsum("nd,nd->n", X64, X64)
    colsum = X64.sum(axis=0)
    sum_l2 = 2.0 * N * sq.sum() - 2.0 * (colsum @ colsum)
    bw = sum_l2 / (N * N - N) / (2.0 ** (NQ // 2))
    cq = np.array([1.0 / (bw * 2.0**q) for q in range(NQ)])  # [5]

    sl = np.asarray(source_label, np.float64)
    tl = np.asarray(target_logits, np.float64)
    ssum = sl.sum(0)
    s_norm = np.where(ssum > 0, sl / np.where(ssum > 0, ssum, 1.0), 0.0)
    tsum = tl.sum(0)
    t_norm = np.where(tsum > 0, tl / np.where(tsum > 0, tsum, 1.0), 0.0)
    s_pres = np.zeros(C)
    np.add.at(s_pres, sl.argmax(1), 1.0)
    t_pres = np.zeros(C)
    np.add.at(t_pres, tl.argmax(1), 1.0)
    common = ((s_pres > 0) & (t_pres > 0)).astype(np.float64)
    V = np.concatenate([s_norm * common, -t_norm * common], axis=0)  # [N, C]

    fq = np.exp(-np.outer(cq, sq))                        # [5, N]

    X8 = X.astype(F8NP)                                   # [N, D]
    xt8 = np.ascontiguousarray(
        X8.T.reshape(NKC, 128, N).transpose(1, 0, 2)      # [128, 8, N]
    )
    return X, sq, cq, V, fq, xt8


def _core_inputs(c, cq, V, fq, xt8):
    gtiles = [(4 * c + s) % 32 for s in S0]
    xtb = np.ascontiguousarray(
        xt8.reshape(128, NKC, NT, 128)[:, :, gtiles, :].reshape(128, NKC, M * 128)
    )
    # vt2[q, t] = 2 V f_q at global tile gtiles[t]
    Vt = V.reshape(NT, 128, C)[gtiles]                    # [M, 128, C]
    fqt = fq.reshape(NQ, NT, 128)[:, gtiles]              # [NQ, M, 128]
    Vq = Vt[None] * fqt[..., None]                        # [NQ, M, 128, C]
    vt2 = (2.0 * Vq).transpose(2, 0, 1, 3).reshape(128, NQ * M * C)
    # vt1 slots: 0..3 = d16 jobs (j = position 8+... tile 16+x), 4..7 = diag x
    vt1 = np.zeros((128, NQ, 8, C))
    for x in range(4):
        jpos = S0.index(x + 16)
        vt1[:, :, x, :] = Vq[:, jpos].transpose(1, 0, 2)
        vt1[:, :, 4 + x, :] = Vq[:, S0.index(x)].transpose(1, 0, 2)
    vt1 = vt1.reshape(128, NQ * 8 * C)
    scl = np.zeros((128, 8))
    for k in range(NQ):
        scl[:, k] = 2.0 * cq[4 - k]
    btab = np.ascontiguousarray(
        np.concatenate([vt2, vt1, scl], axis=1)
    ).astype(BFNP)
    return {"xtb": xtb, "btab": btab}


def _postprocess(results, sq, cq, V):
    # loss = 1/12 sum_q sum_i alpha_q[i] * (sum_cls V[i,cls] R_q[i,cls])
    loss = 0.0
    for c in range(NCORES):
        r = np.asarray(results[c]["r_out"], np.float64).reshape(128, NQ, NI, C)
        for s in range(NI):
            gt_ = (4 * c + S0[s]) % 32
            gi = gt_ * 128 + np.arange(128)
            alpha = np.exp(-np.outer(cq, sq[gi]))         # [NQ, 128]
            loss += np.einsum("qp,pc,pqc->", alpha, V[gi], r[:, :, s, :])
    return loss / C


def _run(in_maps, trace=False, **kw):
    global _BUILT
    if _BUILT is None:
        _BUILT = _build_program()
    return run_bass_kernel_spmd(_BUILT, in_maps, list(range(NCORES)), trace=trace, **kw)


def kernel(source, target, source_label, target_logits, _trace=False, _ret_bkr=False):
    X, sq, cq, V, fq, xt8 = _prep(source, target, source_label, target_logits)
    in_maps = [_core_inputs(c, cq, V, fq, xt8) for c in range(NCORES)]
    bkr = None
    for attempt in range(3):
        try:
            bkr = _run(in_maps, trace=_trace)
            break
        except Exception:
            if attempt == 2:
                raise
            import time as _time

            _time.sleep(2.0)
    loss = _postprocess(bkr.results, sq, cq, V)
    out = np.float32(loss)
    if _ret_bkr:
        return out, bkr
    return out


# revision 59
# speedup vs baseline: 1.0247x; 1.0195x over previous
"""LMMD (DSAN local MMD) loss on 8 Trainium2 NeuronCores — cyclic-support V5.

Math (reference):
    X = concat(source, target)                    # [N=4096, D=1024]
    l2[i,j] = max(|x_i|^2 + |x_j|^2 - 2 x_i.x_j, 0)
    bw      = sum(l2) / (N^2 - N) / 4
    K       = sum_q exp(-l2 / (bw * 2^q)),  q = 0..4
    loss    = sum_c v_c^T K v_c / 12,  V = [s_norm; -t_norm]  (rank-12 weights)

V5 design:
  * Cyclic 16-tile support: core c holds X columns for tiles
    (4c + S0) mod 32 with S0 = {0..7, 12..19}.  The 528 unordered
    128-tile pairs partition into 8 identical 68-job lists (60 weight-2
    oriented pairs covering every (difference, residue) cell once, 4
    weight-1 distance-16 jobs computed twice with opposite orientations,
    4 weight-1 diagonals), so every core runs the SAME program on a
    rotated tile set and per-core X DMA halves to 16 KB/partition.
  * Jobs stream through 9 batches (6|8x7|6 jobs).  Per batch: fp8
    DoubleRow gram into one 2-bank PSUM tile, three bias-free ACT exps
    (e4 = exp(2c4 G) full width, e1/e0 heads straight from the gram at
    8x/16x scale), DVE squaring e3/e2/e1-tail, Pool squares the e0 tail
    from DVE's e1 region only (fully decoupled engine chains).  The
    j-side RBF factor exp(-c_q sq_j) is folded into per-q bf16 vt
    tables; the i-side factor is applied on the host.
  * Weighted reduce keeps es stationary (12-wide moving vt), accumulating
    R_q[i, cls] into two PSUM tiles (q>=2 / q<=1) so the high-q drain
    overlaps the low-q matmuls.  PSUM has_written semantics: one
    start per bank, first-touch overwrites via cleared bits.
"""

import numpy as np
import ml_dtypes

import concourse.bass as bass
from concourse import bacc
import concourse.mybir as mybir
import concourse.tile as tile
from concourse.bass_utils import run_bass_kernel_spmd

B = 2048
D = 1024
C = 12
NCORES = 8
N = 2 * B                 # 4096 total samples
NT = N // 128             # 32 j-tiles
NKC = D // 128            # 8 contraction chunks
NKP = NKC // 2            # 4 DoubleRow chunk-pairs
NQ = 5
M = 16                    # tiles in the cyclic support
NI = 8                    # i-side slots (positions 0..7)
WLAG = 3                  # batches of lag between es production and weighted

S0 = list(range(0, 8)) + list(range(12, 20))

# btab layout (bf16): vt2 [5*M*C] | vt1 [5*8*C] | scales [8]
# scales: [2c4, 2c3, 2c2, 2c1, 2c0, 0(bias), 0, 0]
VT2_COLS = NQ * M * C
VT1_COLS = NQ * 8 * C
NSCL = 8
BT_COLS = VT2_COLS + VT1_COLS + NSCL

F8NP = ml_dtypes.float8_e4m3
BFNP = ml_dtypes.bfloat16

_BUILT = None


def _plan_jobs():
    """Deterministic job plan: 68 (jpos, ipos, weight, vt1slot) tuples in
    S0-local positions, every global pair covered exactly once."""
    import itertools

    Sset = set(S0)
    pos = {t: i for i, t in enumerate(S0)}
    ILOCAL = set(range(0, 8))
    pairs = []
    for a, b in itertools.combinations(S0, 2):
        if a not in ILOCAL and b not in ILOCAL:
            continue
        d = (b - a) % 32
        cells = set()
        for (base, dd) in ((a, d), (b, (32 - d) % 32)):
            if 1 <= dd <= 15:
                cells.add((dd, base % 4))
        if cells:
            pairs.append(((a, b), sorted(cells)))
    cells_needed = [(d, r) for d in range(1, 16) for r in range(4)]
    cell_idx = {c: i for i, c in enumerate(cells_needed)}
    adj = [[] for _ in cells_needed]
    for pi, (fs, cells) in enumerate(pairs):
        for cc in cells:
            if cc in cell_idx:
                adj[cell_idx[cc]].append(pi)
    for ci in range(len(adj)):
        adj[ci].sort(key=lambda pi: max(pairs[pi][0]))
    match_pair = {}
    match_cell = [None] * len(cells_needed)

    def aug(ci, seen):
        for pi in adj[ci]:
            if pi in seen:
                continue
            seen.add(pi)
            if pi not in match_pair or aug(match_pair[pi], seen):
                match_pair[pi] = ci
                match_cell[ci] = pi
                return True
        return False

    for ci in range(len(cells_needed)):
        assert aug(ci, set())
    jobs = []
    for ci, pi in enumerate(match_cell):
        (a, b) = pairs[pi][0]
        i_t = a if a in ILOCAL else b
        j_t = b if i_t == a else a
        jobs.append((pos[j_t], pos[i_t], 2, -1))
    for x in range(4):                       # d16, weight 1, computed twice
        jobs.append((pos[x + 16], pos[x], 1, x))
    for x in range(4):                       # diagonal, weight 1
        jobs.append((pos[x], pos[x], 1, 4 + x))
    # order by data arrival (4-position DMA chunks), then j for locality
    jobs.sort(key=lambda jb: (max(jb[0] // 4, jb[1] // 4), jb[0], jb[1]))
    return jobs


JOBS = _plan_jobs()
BATCH_SIZES = [6, 8, 8, 8, 8, 8, 8, 8, 4, 2]
assert sum(BATCH_SIZES) == len(JOBS) == 68
NPOOL_FREE = 3            # trailing batches whose e0 tail runs on DVE, not Pool
BMAX = max(BATCH_SIZES) * 128
BATCHES = []
_k = 0
for bs in BATCH_SIZES:
    BATCHES.append(JOBS[_k : _k + bs])
    _k += bs
NB = len(BATCHES)


def _asplit(w):
    # ACT's exp(8sG)/exp(16sG) head widths (e1/e0); Pool squares e0[a:]
    if w <= 256:
        return 64
    return 128 if w <= 768 else 192


def _build_program():
    fp32 = mybir.dt.float32
    bf16 = mybir.dt.bfloat16
    f8 = mybir.dt.float8e4
    Exp = mybir.ActivationFunctionType.Exp
    Copy = mybir.ActivationFunctionType.Copy
    DR = mybir.MatmulPerfMode.DoubleRow

    nc = bacc.Bacc()
    # host-pretransposed: xtb[p, k, t*128+j] = X[(4c+S0[t])*128+j, k*128+p]
    xtb = nc.declare_dram_parameter("xtb", [128, NKC, M * 128], f8, isOutput=False)
    btab = nc.declare_dram_parameter("btab", [128, BT_COLS], bf16, isOutput=False)
    rout = nc.declare_dram_parameter("r_out", [128, NQ * NI * C], fp32, isOutput=True)

    with tile.TileContext(nc) as tc:
        with (
            tc.tile_pool(name="singles", bufs=1) as singles,
            tc.tile_pool(name="epool", bufs=6) as epool,
            tc.tile_pool(name="gpsum", bufs=3, space="PSUM") as gpsum,
            tc.tile_pool(name="rqpsum", bufs=1, space="PSUM") as rqpsum,
        ):
            xtb_sb = singles.tile([128, NKC, M * 128], f8)
            btab_sb = singles.tile([128, BT_COLS], bf16)
            # PE p-state warm-up: ~3us of dummy matmuls on a never-written
            # scratch tile so the first real gram runs at full clock.  The
            # results land in a recycled gpsum generation nobody reads.
            # p-state warm-up: ~12 medium matmuls end just before the first
            # gram's data lands, so the real grams run at full clock without
            # the warm-up itself delaying them.
            wsrc = singles.tile([128, 2, 512], f8)
            nc.gpsimd.memset(wsrc[:, :, 0:128], 0.0)
            nc.gpsimd.memset(wsrc[:, :, 128:512], 0.0)
            wu = gpsum.tile([128, BMAX], fp32, tag="g", name="gwarm")
            for k in range(12):
                nc.tensor.matmul(
                    wu[:, 0:512],
                    lhsT=wsrc[:, :, 0:128],
                    rhs=wsrc,
                    start=(k == 0),
                    stop=(k == 11),
                    perf_mode=DR,
                )
            # DMA stream: first batch's tiles (positions 0-3) in two k-halves
            # so gram m=0,1 starts early; scales early (tiny); remaining tile
            # chunks; the bulk vt table after the second chunk.
            nc.sync.dma_start(out=xtb_sb[:, 0:4, 0:512], in_=xtb[:, 0:4, 0:512])
            nc.sync.dma_start(out=xtb_sb[:, 4:8, 0:512], in_=xtb[:, 4:8, 0:512])
            nc.sync.dma_start(
                out=btab_sb[:, VT2_COLS + VT1_COLS :],
                in_=btab[:, VT2_COLS + VT1_COLS :],
            )
            scl_s = singles.tile([128, NSCL], fp32)
            nc.vector.tensor_copy(scl_s, btab_sb[:, VT2_COLS + VT1_COLS :])
            # Exp-table warm-up reads a const tile so it runs during the DMA
            # head instead of waiting for the scale fetch.
            warm_in = singles.tile([128, 4], fp32)
            nc.gpsimd.memset(warm_in, 0.0)
            warm = singles.tile([128, 4], fp32)
            nc.scalar.activation(warm, warm_in, Exp)
            nc.sync.dma_start(out=xtb_sb[:, :, 512:1024], in_=xtb[:, :, 512:1024])
            nc.sync.dma_start(
                out=btab_sb[:, 0 : VT2_COLS + VT1_COLS],
                in_=btab[:, 0 : VT2_COLS + VT1_COLS],
            )
            nc.sync.dma_start(out=xtb_sb[:, :, 1024:1536], in_=xtb[:, :, 1024:1536])
            nc.sync.dma_start(out=xtb_sb[:, :, 1536:2048], in_=xtb[:, :, 1536:2048])

            # R accumulators: hi = q {4,3,2}, lo = q {1,0}; one bank each
            rq_hi = rqpsum.tile([128, 3 * NI * C], fp32, tag="rqh", name="rq_hi")
            rq_lo = rqpsum.tile([128, 2 * NI * C], fp32, tag="rql", name="rq_lo")

            def rq_slice(q, islot):
                if q >= 2:
                    base = ((q - 2) * NI + islot) * C
                    return rq_hi[:, base : base + C]
                base = (q * NI + islot) * C
                return rq_lo[:, base : base + C]

            first_mm = {"hi": True, "lo": True}

            def emit_one(bi, es, q, jj, stop=False):
                jpos, ipos, w, vt1slot = BATCHES[bi][jj]
                if w == 2:
                    vb = (q * M + jpos) * C
                else:
                    vb = VT2_COLS + (q * 8 + vt1slot) * C
                key = "hi" if q >= 2 else "lo"
                nc.tensor.matmul(
                    rq_slice(q, ipos),
                    lhsT=es[q][:, jj * 128 : (jj + 1) * 128],
                    rhs=btab_sb[:, vb : vb + C],
                    start=first_mm[key],
                    stop=stop,
                )
                first_mm[key] = False

            def emit_main(bi, es):
                # q=4..1 plus the q0 jobs living in ACT's exp head — their es
                # is ready early.  q0 jobs overlapping Pool's slow e0 tail are
                # deferred a batch (emit_q0_tail) so they never park in PE's
                # 4-deep wait queue and block later grams.
                jobs = BATCHES[bi]
                a = _asplit(len(jobs) * 128)
                for q in range(NQ - 1, 0, -1):
                    for jj in range(len(jobs)):
                        emit_one(bi, es, q, jj,
                                 stop=(bi == NB - 1 and q == 2 and jj == len(jobs) - 1))
                for jj in range(len(jobs)):
                    if (jj + 1) * 128 <= a:
                        emit_one(bi, es, 0, jj)

            def emit_q0_tail(bi, es, stop=False):
                jobs = BATCHES[bi]
                a = _asplit(len(jobs) * 128)
                late = [jj for jj in range(len(jobs)) if (jj + 1) * 128 > a]
                for k, jj in enumerate(late):
                    emit_one(bi, es, 0, jj, stop=(stop and k == len(late) - 1))

            pending = []
            pending2 = []
            for bi, jobs in enumerate(BATCHES):
                w = len(jobs) * 128
                a = _asplit(w)
                gt = gpsum.tile([128, BMAX], fp32, tag="g", name=f"g{bi}")
                for jj, (jpos, ipos, _, _) in enumerate(jobs):
                    for m in range(NKP):
                        nc.tensor.matmul(
                            gt[:, jj * 128 : (jj + 1) * 128],
                            lhsT=xtb_sb[:, 2 * m : 2 * m + 2, jpos * 128 : (jpos + 1) * 128],
                            rhs=xtb_sb[:, 2 * m : 2 * m + 2, ipos * 128 : (ipos + 1) * 128],
                            start=(m == 0),
                            stop=(m == NKP - 1),
                            perf_mode=DR,
                        )
                es = {q: epool.tile([128, BMAX], bf16, tag=f"e{q}", name=f"e{q}b{bi}") for q in range(NQ)}
                zb = scl_s[:, 5:6]
                nc.scalar.activation(
                    es[4][:, 0:w], gt[:, 0:w], Exp, bias=zb, scale=scl_s[:, 0:1],
                )
                nc.scalar.activation(
                    es[1][:, 0:a], gt[:, 0:a], Exp, bias=zb, scale=scl_s[:, 3:4],
                )
                nc.scalar.activation(
                    es[0][:, 0:a], gt[:, 0:a], Exp, bias=zb, scale=scl_s[:, 4:5],
                )
                nc.vector.tensor_mul(es[3][:, 0:w], es[4][:, 0:w], es[4][:, 0:w])
                nc.vector.tensor_mul(es[2][:, 0:w], es[3][:, 0:w], es[3][:, 0:w])
                if a < w:
                    nc.vector.tensor_mul(es[1][:, a:w], es[2][:, a:w], es[2][:, a:w])
                    if bi >= NB - NPOOL_FREE:
                        nc.vector.tensor_mul(es[0][:, a:w], es[1][:, a:w], es[1][:, a:w])
                    else:
                        nc.gpsimd.tensor_mul(es[0][:, a:w], es[1][:, a:w], es[1][:, a:w])
                pending.append((bi, es))
                if len(pending) > WLAG:
                    item = pending.pop(0)
                    emit_main(*item)
                    pending2.append(item)
                if len(pending2) > 1:
                    emit_q0_tail(*pending2.pop(0))
            for item in pending:
                emit_main(*item)
                pending2.append(item)
            for k, item in enumerate(pending2):
                emit_q0_tail(*item, stop=(k == len(pending2) - 1))

            # tail: hi drains on DVE while the low-q matmuls still run; lo on
            # the by-then-idle ACT; one bf16 DMA.
            stg = singles.tile([128, NQ * NI * C], fp32)
            nc.vector.tensor_copy(stg[:, 2 * NI * C :], rq_hi)
            nc.scalar.activation(stg[:, 0 : 2 * NI * C], rq_lo, Copy)
            nc.sync.dma_start(out=rout[:], in_=stg)

    nc.compile()
    return nc


def _prep(source, target, source_label, target_logits):
    X = np.concatenate([np.asarray(source), np.asarray(target)], axis=0)
    X64 = X.astype(np.float64)
    sq = np.einsum("nd,nd->n", X64, X64)
    colsum = X64.sum(axis=0)
    sum_l2 = 2.0 * N * sq.sum() - 2.0 * (colsum @ colsum)
    bw = sum_l2 / (N * N - N) / (2.0 ** (NQ // 2))
    cq = np.array([1.0 / (bw * 2.0**q) for q in range(NQ)])  # [5]

    sl = np.asarray(source_label, np.float64)
    tl = np.asarray(target_logits, np.float64)
    ssum = sl.sum(0)
    s_norm = np.where(ssum > 0, sl / np.where(ssum > 0, ssum, 1.0), 0.0)
    tsum = tl.sum(0)
    t_norm = np.where(tsum > 0, tl / np.where(tsum > 0, tsum, 1.0), 0.0)
    s_pres = np.zeros(C)
    np.add.at(s_pres, sl.argmax(1), 1.0)
    t_pres = np.zeros(C)
    np.add.at(t_pres, tl.argmax(1), 1.0)
    common = ((s_pres > 0) & (t_pres > 0)).astype(np.float64)
    V = np.concatenate([s_norm * common, -t_norm * common], axis=0)  # [N, C]

    fq = np.exp(-np.outer(cq, sq))                        # [5, N]

    X8 = X.astype(F8NP)                                   # [N, D]
    xt8 = np.ascontiguousarray(
        X8.T.reshape(NKC, 128, N).transpose(1, 0, 2)      # [128, 8, N]
    )
    return X, sq, cq, V, fq, xt8


def _core_inputs(c, cq, V, fq, xt8):
    gtiles = [(4 * c + s) % 32 for s in S0]
    xtb = np.ascontiguousarray(
        xt8.reshape(128, NKC, NT, 128)[:, :, gtiles, :].reshape(128, NKC, M * 128)
    )
    # vt2[q, t] = 2 V f_q at global tile gtiles[t]
    Vt = V.reshape(NT, 128, C)[gtiles]                    # [M, 128, C]
    fqt = fq.reshape(NQ, NT, 128)[:, gtiles]              # [NQ, M, 128]
    Vq = Vt[None] * fqt[..., None]                        # [NQ, M, 128, C]
    vt2 = (2.0 * Vq).transpose(2, 0, 1, 3).reshape(128, NQ * M * C)
    # vt1 slots: 0..3 = d16 jobs (j = position 8+... tile 16+x), 4..7 = diag x
    vt1 = np.zeros((128, NQ, 8, C))
    for x in range(4):
        jpos = S0.index(x + 16)
        vt1[:, :, x, :] = Vq[:, jpos].transpose(1, 0, 2)
        vt1[:, :, 4 + x, :] = Vq[:, S0.index(x)].transpose(1, 0, 2)
    vt1 = vt1.reshape(128, NQ * 8 * C)
    scl = np.zeros((128, 8))
    for k in range(NQ):
        scl[:, k] = 2.0 * cq[4 - k]
    btab = np.ascontiguousarray(
        np.concatenate([vt2, vt1, scl], axis=1)
    ).astype(BFNP)
    return {"xtb": xtb, "btab": btab}


def _postprocess(results, sq, cq, V):
    # loss = 1/12 sum_q sum_i alpha_q[i] * (sum_cls V[i,cls] R_q[i,cls])
    loss = 0.0
    for c in range(NCORES):
        r = np.asarray(results[c]["r_out"], np.float64).reshape(128, NQ, NI, C)
        for s in range(NI):
            gt_ = (4 * c + S0[s]) % 32
            gi = gt_ * 128 + np.arange(128)
            alpha = np.exp(-np.outer(cq, sq[gi]))         # [NQ, 128]
            loss += np.einsum("qp,pc,pqc->", alpha, V[gi], r[:, :, s, :])
    return loss / C


def _run(in_maps, trace=False, **kw):
    global _BUILT
    if _BUILT is None:
        _BUILT = _build_program()
    return run_bass_kernel_spmd(_BUILT, in_maps, list(range(NCORES)), trace=trace, **kw)


def kernel(source, target, source_label, target_logits, _trace=False, _ret_bkr=False):
    X, sq, cq, V, fq, xt8 = _prep(source, target, source_label, target_logits)
    in_maps = [_core_inputs(c, cq, V, fq, xt8) for c in range(NCORES)]
    bkr = None
    for attempt in range(3):
        try:
            bkr = _run(in_maps, trace=_trace)
            break
        except Exception:
            if attempt == 2:
                raise
            import time as _time

            _time.sleep(2.0)
    loss = _postprocess(bkr.results, sq, cq, V)
    out = np.float32(loss)
    if _ret_bkr:
        return out, bkr
    return out


# revision 63
# speedup vs baseline: 1.0454x; 1.0202x over previous
"""LMMD (DSAN local MMD) loss on 8 Trainium2 NeuronCores — cyclic-support V5.

Math (reference):
    X = concat(source, target)                    # [N=4096, D=1024]
    l2[i,j] = max(|x_i|^2 + |x_j|^2 - 2 x_i.x_j, 0)
    bw      = sum(l2) / (N^2 - N) / 4
    K       = sum_q exp(-l2 / (bw * 2^q)),  q = 0..4
    loss    = sum_c v_c^T K v_c / 12,  V = [s_norm; -t_norm]  (rank-12 weights)

V5 design:
  * Cyclic 16-tile support: core c holds X columns for tiles
    (4c + S0) mod 32 with S0 = {0..7, 12..19}.  The 528 unordered
    128-tile pairs partition into 8 identical 68-job lists (60 weight-2
    oriented pairs covering every (difference, residue) cell once, 4
    weight-1 distance-16 jobs computed twice with opposite orientations,
    4 weight-1 diagonals), so every core runs the SAME program on a
    rotated tile set and per-core X DMA halves to 16 KB/partition.
  * Jobs stream through 9 batches (6|8x7|6 jobs).  Per batch: fp8
    DoubleRow gram into one 2-bank PSUM tile, three bias-free ACT exps
    (e4 = exp(2c4 G) full width, e1/e0 heads straight from the gram at
    8x/16x scale), DVE squaring e3/e2/e1-tail, Pool squares the e0 tail
    from DVE's e1 region only (fully decoupled engine chains).  The
    j-side RBF factor exp(-c_q sq_j) is folded into per-q bf16 vt
    tables; the i-side factor is applied on the host.
  * Weighted reduce keeps es stationary (12-wide moving vt), accumulating
    R_q[i, cls] into two PSUM tiles (q>=2 / q<=1) so the high-q drain
    overlaps the low-q matmuls.  PSUM has_written semantics: one
    start per bank, first-touch overwrites via cleared bits.
"""

import numpy as np
import ml_dtypes

import concourse.bass as bass
from concourse import bacc
import concourse.mybir as mybir
import concourse.tile as tile
from concourse.bass_utils import run_bass_kernel_spmd

B = 2048
D = 1024
C = 12
NCORES = 8
N = 2 * B                 # 4096 total samples
NT = N // 128             # 32 j-tiles
NKC = D // 128            # 8 contraction chunks
NKP = NKC // 2            # 4 DoubleRow chunk-pairs
NQ = 5
M = 16                    # tiles in the cyclic support
NI = 8                    # i-side slots (positions 0..7)
WLAG = 3                  # batches of lag between es production and weighted

S0 = list(range(0, 8)) + list(range(12, 20))

# btab layout (bf16): vt2 [5*M*C] | vt1 [5*8*C] | scales [8]
# scales: [2c4, 2c3, 2c2, 2c1, 2c0, 0(bias), 0, 0]
VT2_COLS = NQ * M * C
VT1_COLS = NQ * 8 * C
NSCL = 8
BT_COLS = VT2_COLS + VT1_COLS + NSCL

F8NP = ml_dtypes.float8_e4m3
BFNP = ml_dtypes.bfloat16

_BUILT = None


def _plan_jobs():
    """Deterministic job plan: 68 (jpos, ipos, weight, vt1slot) tuples in
    S0-local positions, every global pair covered exactly once."""
    import itertools

    Sset = set(S0)
    pos = {t: i for i, t in enumerate(S0)}
    ILOCAL = set(range(0, 8))
    pairs = []
    for a, b in itertools.combinations(S0, 2):
        if a not in ILOCAL and b not in ILOCAL:
            continue
        d = (b - a) % 32
        cells = set()
        for (base, dd) in ((a, d), (b, (32 - d) % 32)):
            if 1 <= dd <= 15:
                cells.add((dd, base % 4))
        if cells:
            pairs.append(((a, b), sorted(cells)))
    cells_needed = [(d, r) for d in range(1, 16) for r in range(4)]
    cell_idx = {c: i for i, c in enumerate(cells_needed)}
    adj = [[] for _ in cells_needed]
    for pi, (fs, cells) in enumerate(pairs):
        for cc in cells:
            if cc in cell_idx:
                adj[cell_idx[cc]].append(pi)
    for ci in range(len(adj)):
        adj[ci].sort(key=lambda pi: max(pairs[pi][0]))
    match_pair = {}
    match_cell = [None] * len(cells_needed)

    def aug(ci, seen):
        for pi in adj[ci]:
            if pi in seen:
                continue
            seen.add(pi)
            if pi not in match_pair or aug(match_pair[pi], seen):
                match_pair[pi] = ci
                match_cell[ci] = pi
                return True
        return False

    for ci in range(len(cells_needed)):
        assert aug(ci, set())
    jobs = []
    for ci, pi in enumerate(match_cell):
        (a, b) = pairs[pi][0]
        i_t = a if a in ILOCAL else b
        j_t = b if i_t == a else a
        jobs.append((pos[j_t], pos[i_t], 2, -1))
    for x in range(4):                       # d16, weight 1, computed twice
        jobs.append((pos[x + 16], pos[x], 1, x))
    for x in range(4):                       # diagonal, weight 1
        jobs.append((pos[x], pos[x], 1, 4 + x))
    # order by data arrival (4-position DMA chunks), then j for locality
    jobs.sort(key=lambda jb: (max(jb[0] // 4, jb[1] // 4), jb[0], jb[1]))
    return jobs


JOBS = _plan_jobs()
BATCH_SIZES = [6, 8, 8, 8, 8, 8, 8, 8, 4, 2]
assert sum(BATCH_SIZES) == len(JOBS) == 68
NPOOL_FREE = 3            # trailing batches whose e0 tail runs on DVE, not Pool
BMAX = max(BATCH_SIZES) * 128
BATCHES = []
_k = 0
for bs in BATCH_SIZES:
    BATCHES.append(JOBS[_k : _k + bs])
    _k += bs
NB = len(BATCHES)


def _asplit(w):
    # ACT's exp(8sG)/exp(16sG) head widths (e1/e0); Pool squares e0[a:]
    if w <= 256:
        return 64
    return 128 if w <= 768 else 192


def _build_program():
    fp32 = mybir.dt.float32
    bf16 = mybir.dt.bfloat16
    f8 = mybir.dt.float8e4
    Exp = mybir.ActivationFunctionType.Exp
    Copy = mybir.ActivationFunctionType.Copy
    Square = mybir.ActivationFunctionType.Square
    DR = mybir.MatmulPerfMode.DoubleRow

    nc = bacc.Bacc()
    # host-pretransposed: xtb[p, k, t*128+j] = X[(4c+S0[t])*128+j, k*128+p]
    xtb = nc.declare_dram_parameter("xtb", [128, NKC, M * 128], f8, isOutput=False)
    btab = nc.declare_dram_parameter("btab", [128, BT_COLS], bf16, isOutput=False)
    rout = nc.declare_dram_parameter("r_out", [128, NQ * NI * C], fp32, isOutput=True)

    with tile.TileContext(nc) as tc:
        with (
            tc.tile_pool(name="singles", bufs=1) as singles,
            tc.tile_pool(name="epool", bufs=6) as epool,
            tc.tile_pool(name="gpsum", bufs=3, space="PSUM") as gpsum,
            tc.tile_pool(name="rqpsum", bufs=1, space="PSUM") as rqpsum,
        ):
            xtb_sb = singles.tile([128, NKC, M * 128], f8)
            btab_sb = singles.tile([128, BT_COLS], bf16)
            # PE p-state warm-up: ~3us of dummy matmuls on a never-written
            # scratch tile so the first real gram runs at full clock.  The
            # results land in a recycled gpsum generation nobody reads.
            # p-state warm-up: ~12 medium matmuls end just before the first
            # gram's data lands, so the real grams run at full clock without
            # the warm-up itself delaying them.
            wsrc = singles.tile([128, 2, 512], f8)
            nc.gpsimd.memset(wsrc[:, :, 0:128], 0.0)
            nc.gpsimd.memset(wsrc[:, :, 128:512], 0.0)
            wu = gpsum.tile([128, BMAX], fp32, tag="g", name="gwarm")
            for k in range(12):
                nc.tensor.matmul(
                    wu[:, 0:512],
                    lhsT=wsrc[:, :, 0:128],
                    rhs=wsrc,
                    start=(k == 0),
                    stop=(k == 11),
                    perf_mode=DR,
                )
            # DMA stream: first batch's tiles (positions 0-3) in two k-halves
            # so gram m=0,1 starts early; scales early (tiny); remaining tile
            # chunks; the bulk vt table after the second chunk.
            nc.sync.dma_start(out=xtb_sb[:, 0:4, 0:512], in_=xtb[:, 0:4, 0:512])
            nc.sync.dma_start(out=xtb_sb[:, 4:8, 0:512], in_=xtb[:, 4:8, 0:512])
            nc.sync.dma_start(
                out=btab_sb[:, VT2_COLS + VT1_COLS :],
                in_=btab[:, VT2_COLS + VT1_COLS :],
            )
            scl_s = singles.tile([128, NSCL], fp32)
            nc.vector.tensor_copy(scl_s, btab_sb[:, VT2_COLS + VT1_COLS :])
            # Exp-table warm-up reads a const tile so it runs during the DMA
            # head instead of waiting for the scale fetch.
            warm_in = singles.tile([128, 4], fp32)
            nc.gpsimd.memset(warm_in, 0.0)
            warm = singles.tile([128, 4], fp32)
            nc.scalar.activation(warm, warm_in, Exp)
            nc.sync.dma_start(out=xtb_sb[:, :, 512:1024], in_=xtb[:, :, 512:1024])
            nc.sync.dma_start(
                out=btab_sb[:, 0 : VT2_COLS + VT1_COLS],
                in_=btab[:, 0 : VT2_COLS + VT1_COLS],
            )
            nc.sync.dma_start(out=xtb_sb[:, :, 1024:1536], in_=xtb[:, :, 1024:1536])
            nc.sync.dma_start(out=xtb_sb[:, :, 1536:2048], in_=xtb[:, :, 1536:2048])

            # R accumulators: hi = q {4,3,2}, lo = q {1,0}; one bank each
            rq_hi = rqpsum.tile([128, 3 * NI * C], fp32, tag="rqh", name="rq_hi")
            rq_lo = rqpsum.tile([128, 2 * NI * C], fp32, tag="rql", name="rq_lo")

            def rq_slice(q, islot):
                if q >= 2:
                    base = ((q - 2) * NI + islot) * C
                    return rq_hi[:, base : base + C]
                base = (q * NI + islot) * C
                return rq_lo[:, base : base + C]

            first_mm = {"hi": True, "lo": True}

            def emit_one(bi, es, q, jj, stop=False):
                jpos, ipos, w, vt1slot = BATCHES[bi][jj]
                if w == 2:
                    vb = (q * M + jpos) * C
                else:
                    vb = VT2_COLS + (q * 8 + vt1slot) * C
                key = "hi" if q >= 2 else "lo"
                nc.tensor.matmul(
                    rq_slice(q, ipos),
                    lhsT=es[q][:, jj * 128 : (jj + 1) * 128],
                    rhs=btab_sb[:, vb : vb + C],
                    start=first_mm[key],
                    stop=stop,
                )
                first_mm[key] = False

            def emit_main(bi, es):
                # q=4..1 plus the q0 jobs living in ACT's exp head — their es
                # is ready early.  q0 jobs overlapping Pool's slow e0 tail are
                # deferred a batch (emit_q0_tail) so they never park in PE's
                # 4-deep wait queue and block later grams.
                jobs = BATCHES[bi]
                a = _asplit(len(jobs) * 128)
                for q in range(NQ - 1, 0, -1):
                    for jj in range(len(jobs)):
                        emit_one(bi, es, q, jj,
                                 stop=(bi == NB - 1 and q == 2 and jj == len(jobs) - 1))
                for jj in range(len(jobs)):
                    if (jj + 1) * 128 <= a:
                        emit_one(bi, es, 0, jj)

            def emit_q0_tail(bi, es, stop=False):
                jobs = BATCHES[bi]
                a = _asplit(len(jobs) * 128)
                late = [jj for jj in range(len(jobs)) if (jj + 1) * 128 > a]
                for k, jj in enumerate(late):
                    emit_one(bi, es, 0, jj, stop=(stop and k == len(late) - 1))

            pending = []
            pending2 = []
            for bi, jobs in enumerate(BATCHES):
                w = len(jobs) * 128
                a = _asplit(w)
                # weighted for lagged batches goes BEFORE this gram: the
                # gram's Ldweights park in PE's 4-deep wait queue (PSUM slot
                # not yet free) and would block the ready weighted stream.
                if len(pending) > WLAG:
                    item = pending.pop(0)
                    emit_main(*item)
                    pending2.append(item)
                if len(pending2) > 1:
                    emit_q0_tail(*pending2.pop(0))
                gt = gpsum.tile([128, BMAX], fp32, tag="g", name=f"g{bi}")
                for jj, (jpos, ipos, _, _) in enumerate(jobs):
                    for m in range(NKP):
                        nc.tensor.matmul(
                            gt[:, jj * 128 : (jj + 1) * 128],
                            lhsT=xtb_sb[:, 2 * m : 2 * m + 2, jpos * 128 : (jpos + 1) * 128],
                            rhs=xtb_sb[:, 2 * m : 2 * m + 2, ipos * 128 : (ipos + 1) * 128],
                            start=(m == 0),
                            stop=(m == NKP - 1),
                            perf_mode=DR,
                        )
                es = {q: epool.tile([128, BMAX], bf16, tag=f"e{q}", name=f"e{q}b{bi}") for q in range(NQ)}
                zb = scl_s[:, 5:6]
                # Only e4 reads the gram — its PSUM slot frees immediately,
                # so later grams never stall PE on the pool semaphore.  The
                # e1/e0 heads are ACT Squares chained off es2/es1 (their only
                # consumers are WLAG-slack weighted matmuls + each other).
                nc.scalar.activation(
                    es[4][:, 0:w], gt[:, 0:w], Exp, bias=zb, scale=scl_s[:, 0:1],
                )
                nc.vector.tensor_mul(es[3][:, 0:w], es[4][:, 0:w], es[4][:, 0:w])
                nc.vector.tensor_mul(es[2][:, 0:w], es[3][:, 0:w], es[3][:, 0:w])
                nc.scalar.activation(
                    es[1][:, 0:a], es[2][:, 0:a], Square, bias=zb, scale=1.0,
                )
                nc.scalar.activation(
                    es[0][:, 0:a], es[1][:, 0:a], Square, bias=zb, scale=1.0,
                )
                if a < w:
                    nc.vector.tensor_mul(es[1][:, a:w], es[2][:, a:w], es[2][:, a:w])
                    if bi >= NB - NPOOL_FREE:
                        nc.vector.tensor_mul(es[0][:, a:w], es[1][:, a:w], es[1][:, a:w])
                    else:
                        nc.gpsimd.tensor_mul(es[0][:, a:w], es[1][:, a:w], es[1][:, a:w])
                pending.append((bi, es))
            for item in pending:
                emit_main(*item)
                pending2.append(item)
            for k, item in enumerate(pending2):
                emit_q0_tail(*item, stop=(k == len(pending2) - 1))

            # tail: hi drains on DVE while the low-q matmuls still run; lo on
            # the by-then-idle ACT; one bf16 DMA.
            stg = singles.tile([128, NQ * NI * C], fp32)
            nc.vector.tensor_copy(stg[:, 2 * NI * C :], rq_hi)
            nc.scalar.activation(stg[:, 0 : 2 * NI * C], rq_lo, Copy)
            nc.sync.dma_start(out=rout[:], in_=stg)

    nc.compile()
    return nc


def _prep(source, target, source_label, target_logits):
    X = np.concatenate([np.asarray(source), np.asarray(target)], axis=0)
    X64 = X.astype(np.float64)
    sq = np.einsum("nd,nd->n", X64, X64)
    colsum = X64.sum(axis=0)
    sum_l2 = 2.0 * N * sq.sum() - 2.0 * (colsum @ colsum)
    bw = sum_l2 / (N * N - N) / (2.0 ** (NQ // 2))
    cq = np.array([1.0 / (bw * 2.0**q) for q in range(NQ)])  # [5]

    sl = np.asarray(source_label, np.float64)
    tl = np.asarray(target_logits, np.float64)
    ssum = sl.sum(0)
    s_norm = np.where(ssum > 0, sl / np.where(ssum > 0, ssum, 1.0), 0.0)
    tsum = tl.sum(0)
    t_norm = np.where(tsum > 0, tl / np.where(tsum > 0, tsum, 1.0), 0.0)
    s_pres = np.zeros(C)
    np.add.at(s_pres, sl.argmax(1), 1.0)
    t_pres = np.zeros(C)
    np.add.at(t_pres, tl.argmax(1), 1.0)
    common = ((s_pres > 0) & (t_pres > 0)).astype(np.float64)
    V = np.concatenate([s_norm * common, -t_norm * common], axis=0)  # [N, C]

    fq = np.exp(-np.outer(cq, sq))                        # [5, N]

    X8 = X.astype(F8NP)                                   # [N, D]
    xt8 = np.ascontiguousarray(
        X8.T.reshape(NKC, 128, N).transpose(1, 0, 2)      # [128, 8, N]
    )
    return X, sq, cq, V, fq, xt8


def _core_inputs(c, cq, V, fq, xt8):
    gtiles = [(4 * c + s) % 32 for s in S0]
    xtb = np.ascontiguousarray(
        xt8.reshape(128, NKC, NT, 128)[:, :, gtiles, :].reshape(128, NKC, M * 128)
    )
    # vt2[q, t] = 2 V f_q at global tile gtiles[t]
    Vt = V.reshape(NT, 128, C)[gtiles]                    # [M, 128, C]
    fqt = fq.reshape(NQ, NT, 128)[:, gtiles]              # [NQ, M, 128]
    Vq = Vt[None] * fqt[..., None]                        # [NQ, M, 128, C]
    vt2 = (2.0 * Vq).transpose(2, 0, 1, 3).reshape(128, NQ * M * C)
    # vt1 slots: 0..3 = d16 jobs (j = position 8+... tile 16+x), 4..7 = diag x
    vt1 = np.zeros((128, NQ, 8, C))
    for x in range(4):
        jpos = S0.index(x + 16)
        vt1[:, :, x, :] = Vq[:, jpos].transpose(1, 0, 2)
        vt1[:, :, 4 + x, :] = Vq[:, S0.index(x)].transpose(1, 0, 2)
    vt1 = vt1.reshape(128, NQ * 8 * C)
    scl = np.zeros((128, 8))
    for k in range(NQ):
        scl[:, k] = 2.0 * cq[4 - k]
    btab = np.ascontiguousarray(
        np.concatenate([vt2, vt1, scl], axis=1)
    ).astype(BFNP)
    return {"xtb": xtb, "btab": btab}


def _postprocess(results, sq, cq, V):
    # loss = 1/12 sum_q sum_i alpha_q[i] * (sum_cls V[i,cls] R_q[i,cls])
    loss = 0.0
    for c in range(NCORES):
        r = np.asarray(results[c]["r_out"], np.float64).reshape(128, NQ, NI, C)
        for s in range(NI):
            gt_ = (4 * c + S0[s]) % 32
            gi = gt_ * 128 + np.arange(128)
            alpha = np.exp(-np.outer(cq, sq[gi]))         # [NQ, 128]
            loss += np.einsum("qp,pc,pqc->", alpha, V[gi], r[:, :, s, :])
    return loss / C


def _run(in_maps, trace=False, **kw):
    global _BUILT
    if _BUILT is None:
        _BUILT = _build_program()
    return run_bass_kernel_spmd(_BUILT, in_maps, list(range(NCORES)), trace=trace, **kw)


def kernel(source, target, source_label, target_logits, _trace=False, _ret_bkr=False):
    X, sq, cq, V, fq, xt8 = _prep(source, target, source_label, target_logits)
    in_maps = [_core_inputs(c, cq, V, fq, xt8) for c in range(NCORES)]
    bkr = None
    for attempt in range(3):
        try:
            bkr = _run(in_maps, trace=_trace)
            break
        except Exception:
            if attempt == 2:
                raise
            import time as _time

            _time.sleep(2.0)
    loss = _postprocess(bkr.results, sq, cq, V)
    out = np.float32(loss)
    if _ret_bkr:
        return out, bkr
    return out


# revision 66
# speedup vs baseline: 1.0593x; 1.0133x over previous
"""LMMD (DSAN local MMD) loss on 8 Trainium2 NeuronCores — cyclic-support V5.

Math (reference):
    X = concat(source, target)                    # [N=4096, D=1024]
    l2[i,j] = max(|x_i|^2 + |x_j|^2 - 2 x_i.x_j, 0)
    bw      = sum(l2) / (N^2 - N) / 4
    K       = sum_q exp(-l2 / (bw * 2^q)),  q = 0..4
    loss    = sum_c v_c^T K v_c / 12,  V = [s_norm; -t_norm]  (rank-12 weights)

V5 design:
  * Cyclic 16-tile support: core c holds X columns for tiles
    (4c + S0) mod 32 with S0 = {0..7, 12..19}.  The 528 unordered
    128-tile pairs partition into 8 identical 68-job lists (60 weight-2
    oriented pairs covering every (difference, residue) cell once, 4
    weight-1 distance-16 jobs computed twice with opposite orientations,
    4 weight-1 diagonals), so every core runs the SAME program on a
    rotated tile set and per-core X DMA halves to 16 KB/partition.
  * Jobs stream through 9 batches (6|8x7|6 jobs).  Per batch: fp8
    DoubleRow gram into one 2-bank PSUM tile, three bias-free ACT exps
    (e4 = exp(2c4 G) full width, e1/e0 heads straight from the gram at
    8x/16x scale), DVE squaring e3/e2/e1-tail, Pool squares the e0 tail
    from DVE's e1 region only (fully decoupled engine chains).  The
    j-side RBF factor exp(-c_q sq_j) is folded into per-q bf16 vt
    tables; the i-side factor is applied on the host.
  * Weighted reduce keeps es stationary (12-wide moving vt), accumulating
    R_q[i, cls] into two PSUM tiles (q>=2 / q<=1) so the high-q drain
    overlaps the low-q matmuls.  PSUM has_written semantics: one
    start per bank, first-touch overwrites via cleared bits.
"""

import numpy as np
import ml_dtypes

import concourse.bass as bass
from concourse import bacc
import concourse.mybir as mybir
import concourse.tile as tile
from concourse.bass_utils import run_bass_kernel_spmd

B = 2048
D = 1024
C = 12
NCORES = 8
N = 2 * B                 # 4096 total samples
NT = N // 128             # 32 j-tiles
NKC = D // 128            # 8 contraction chunks
NKP = NKC // 2            # 4 DoubleRow chunk-pairs
NQ = 5
M = 16                    # tiles in the cyclic support
NI = 8                    # i-side slots (positions 0..7)
WLAG = 3                  # batches of lag between es production and weighted

S0 = list(range(0, 8)) + list(range(12, 20))

# btab layout (bf16): vt2 [5*M*C] | vt1 [5*8*C] | scales [8]
# scales: [2c4, 2c3, 2c2, 2c1, 2c0, 0(bias), 0, 0]
VT2_COLS = NQ * M * C
VT1_COLS = NQ * 8 * C
NSCL = 8
BT_COLS = VT2_COLS + VT1_COLS + NSCL

F8NP = ml_dtypes.float8_e4m3
BFNP = ml_dtypes.bfloat16

_BUILT = None


def _plan_jobs():
    """Deterministic job plan: 68 (jpos, ipos, weight, vt1slot) tuples in
    S0-local positions, every global pair covered exactly once."""
    import itertools

    Sset = set(S0)
    pos = {t: i for i, t in enumerate(S0)}
    ILOCAL = set(range(0, 8))
    pairs = []
    for a, b in itertools.combinations(S0, 2):
        if a not in ILOCAL and b not in ILOCAL:
            continue
        d = (b - a) % 32
        cells = set()
        for (base, dd) in ((a, d), (b, (32 - d) % 32)):
            if 1 <= dd <= 15:
                cells.add((dd, base % 4))
        if cells:
            pairs.append(((a, b), sorted(cells)))
    cells_needed = [(d, r) for d in range(1, 16) for r in range(4)]
    cell_idx = {c: i for i, c in enumerate(cells_needed)}
    adj = [[] for _ in cells_needed]
    for pi, (fs, cells) in enumerate(pairs):
        for cc in cells:
            if cc in cell_idx:
                adj[cell_idx[cc]].append(pi)
    for ci in range(len(adj)):
        adj[ci].sort(key=lambda pi: max(pairs[pi][0]))
    match_pair = {}
    match_cell = [None] * len(cells_needed)

    def aug(ci, seen):
        for pi in adj[ci]:
            if pi in seen:
                continue
            seen.add(pi)
            if pi not in match_pair or aug(match_pair[pi], seen):
                match_pair[pi] = ci
                match_cell[ci] = pi
                return True
        return False

    for ci in range(len(cells_needed)):
        assert aug(ci, set())
    jobs = []
    for ci, pi in enumerate(match_cell):
        (a, b) = pairs[pi][0]
        i_t = a if a in ILOCAL else b
        j_t = b if i_t == a else a
        jobs.append((pos[j_t], pos[i_t], 2, -1))
    for x in range(4):                       # d16, weight 1, computed twice
        jobs.append((pos[x + 16], pos[x], 1, x))
    for x in range(4):                       # diagonal, weight 1
        jobs.append((pos[x], pos[x], 1, 4 + x))
    # order by data arrival (4-position DMA chunks), then j for locality
    jobs.sort(key=lambda jb: (max(jb[0] // 4, jb[1] // 4), jb[0], jb[1]))
    return jobs


JOBS = _plan_jobs()
BATCH_SIZES = [10, 12, 12, 12, 12, 6, 4]
assert sum(BATCH_SIZES) == len(JOBS) == 68
NPOOL_FREE = 2            # trailing batches whose e0 tail runs on DVE, not Pool
BMAX = max(BATCH_SIZES) * 128
BATCHES = []
_k = 0
for bs in BATCH_SIZES:
    BATCHES.append(JOBS[_k : _k + bs])
    _k += bs
NB = len(BATCHES)


def _asplit(w):
    # ACT's Square head widths (e1/e0); Pool squares e0[a:]
    if w <= 256:
        return 64
    if w <= 768:
        return 128
    return 288 if w <= 1280 else 352


def _build_program():
    fp32 = mybir.dt.float32
    bf16 = mybir.dt.bfloat16
    f8 = mybir.dt.float8e4
    Exp = mybir.ActivationFunctionType.Exp
    Copy = mybir.ActivationFunctionType.Copy
    Square = mybir.ActivationFunctionType.Square
    DR = mybir.MatmulPerfMode.DoubleRow

    nc = bacc.Bacc()
    # host-pretransposed: xtb[p, k, t*128+j] = X[(4c+S0[t])*128+j, k*128+p]
    xtb = nc.declare_dram_parameter("xtb", [128, NKC, M * 128], f8, isOutput=False)
    btab = nc.declare_dram_parameter("btab", [128, BT_COLS], bf16, isOutput=False)
    rout = nc.declare_dram_parameter("r_out", [128, NQ * NI * C], fp32, isOutput=True)

    with tile.TileContext(nc) as tc:
        with (
            tc.tile_pool(name="singles", bufs=1) as singles,
            tc.tile_pool(name="epool", bufs=6) as epool,
            tc.tile_pool(name="gpsum", bufs=2, space="PSUM") as gpsum,
            tc.tile_pool(name="rqpsum", bufs=1, space="PSUM") as rqpsum,
        ):
            xtb_sb = singles.tile([128, NKC, M * 128], f8)
            btab_sb = singles.tile([128, BT_COLS], bf16)
            # PE p-state warm-up: ~3us of dummy matmuls on a never-written
            # scratch tile so the first real gram runs at full clock.  The
            # results land in a recycled gpsum generation nobody reads.
            # p-state warm-up: ~12 medium matmuls end just before the first
            # gram's data lands, so the real grams run at full clock without
            # the warm-up itself delaying them.
            wsrc = singles.tile([128, 2, 512], f8)
            nc.gpsimd.memset(wsrc[:, :, 0:128], 0.0)
            nc.gpsimd.memset(wsrc[:, :, 128:512], 0.0)
            wu = gpsum.tile([128, BMAX], fp32, tag="g", name="gwarm")
            for k in range(12):
                nc.tensor.matmul(
                    wu[:, 0:512],
                    lhsT=wsrc[:, :, 0:128],
                    rhs=wsrc,
                    start=(k == 0),
                    stop=(k == 11),
                    perf_mode=DR,
                )
            # DMA stream: first batch's tiles (positions 0-3) in two k-halves
            # so gram m=0,1 starts early; scales early (tiny); remaining tile
            # chunks; the bulk vt table after the second chunk.
            nc.sync.dma_start(out=xtb_sb[:, 0:4, 0:512], in_=xtb[:, 0:4, 0:512])
            nc.sync.dma_start(out=xtb_sb[:, 4:8, 0:512], in_=xtb[:, 4:8, 0:512])
            nc.sync.dma_start(
                out=btab_sb[:, VT2_COLS + VT1_COLS :],
                in_=btab[:, VT2_COLS + VT1_COLS :],
            )
            scl_s = singles.tile([128, NSCL], fp32)
            nc.vector.tensor_copy(scl_s, btab_sb[:, VT2_COLS + VT1_COLS :])
            # Exp-table warm-up reads a const tile so it runs during the DMA
            # head instead of waiting for the scale fetch.
            warm_in = singles.tile([128, 4], fp32)
            nc.gpsimd.memset(warm_in, 0.0)
            warm = singles.tile([128, 4], fp32)
            nc.scalar.activation(warm, warm_in, Exp)
            nc.sync.dma_start(out=xtb_sb[:, :, 512:1024], in_=xtb[:, :, 512:1024])
            nc.sync.dma_start(
                out=btab_sb[:, 0 : VT2_COLS + VT1_COLS],
                in_=btab[:, 0 : VT2_COLS + VT1_COLS],
            )
            nc.sync.dma_start(out=xtb_sb[:, :, 1024:1536], in_=xtb[:, :, 1024:1536])
            nc.sync.dma_start(out=xtb_sb[:, :, 1536:2048], in_=xtb[:, :, 1536:2048])

            # R accumulators: hi = q {4,3,2}, lo = q {1,0}; one bank each
            rq_hi = rqpsum.tile([128, 3 * NI * C], fp32, tag="rqh", name="rq_hi")
            rq_lo = rqpsum.tile([128, 2 * NI * C], fp32, tag="rql", name="rq_lo")

            def rq_slice(q, islot):
                if q >= 2:
                    base = ((q - 2) * NI + islot) * C
                    return rq_hi[:, base : base + C]
                base = (q * NI + islot) * C
                return rq_lo[:, base : base + C]

            first_mm = {"hi": True, "lo": True}

            def emit_one(bi, es, q, jj, stop=False):
                jpos, ipos, w, vt1slot = BATCHES[bi][jj]
                if w == 2:
                    vb = (q * M + jpos) * C
                else:
                    vb = VT2_COLS + (q * 8 + vt1slot) * C
                key = "hi" if q >= 2 else "lo"
                nc.tensor.matmul(
                    rq_slice(q, ipos),
                    lhsT=es[q][:, jj * 128 : (jj + 1) * 128],
                    rhs=btab_sb[:, vb : vb + C],
                    start=first_mm[key],
                    stop=stop,
                )
                first_mm[key] = False

            def emit_main(bi, es):
                # q=4..1 plus the q0 jobs living in ACT's exp head — their es
                # is ready early.  q0 jobs overlapping Pool's slow e0 tail are
                # deferred a batch (emit_q0_tail) so they never park in PE's
                # 4-deep wait queue and block later grams.
                jobs = BATCHES[bi]
                a = _asplit(len(jobs) * 128)
                for q in range(NQ - 1, 0, -1):
                    for jj in range(len(jobs)):
                        emit_one(bi, es, q, jj,
                                 stop=(bi == NB - 1 and q == 2 and jj == len(jobs) - 1))
                for jj in range(len(jobs)):
                    if (jj + 1) * 128 <= a:
                        emit_one(bi, es, 0, jj)

            def emit_q0_tail(bi, es, stop=False):
                jobs = BATCHES[bi]
                a = _asplit(len(jobs) * 128)
                late = [jj for jj in range(len(jobs)) if (jj + 1) * 128 > a]
                for k, jj in enumerate(late):
                    emit_one(bi, es, 0, jj, stop=(stop and k == len(late) - 1))

            pending = []
            pending2 = []
            for bi, jobs in enumerate(BATCHES):
                w = len(jobs) * 128
                a = _asplit(w)
                # weighted for lagged batches goes BEFORE this gram: the
                # gram's Ldweights park in PE's 4-deep wait queue (PSUM slot
                # not yet free) and would block the ready weighted stream.
                if len(pending) > WLAG:
                    item = pending.pop(0)
                    emit_main(*item)
                    pending2.append(item)
                if len(pending2) > 1:
                    emit_q0_tail(*pending2.pop(0))
                gt = gpsum.tile([128, BMAX], fp32, tag="g", name=f"g{bi}")
                for jj, (jpos, ipos, _, _) in enumerate(jobs):
                    for m in range(NKP):
                        nc.tensor.matmul(
                            gt[:, jj * 128 : (jj + 1) * 128],
                            lhsT=xtb_sb[:, 2 * m : 2 * m + 2, jpos * 128 : (jpos + 1) * 128],
                            rhs=xtb_sb[:, 2 * m : 2 * m + 2, ipos * 128 : (ipos + 1) * 128],
                            start=(m == 0),
                            stop=(m == NKP - 1),
                            perf_mode=DR,
                        )
                es = {q: epool.tile([128, BMAX], bf16, tag=f"e{q}", name=f"e{q}b{bi}") for q in range(NQ)}
                zb = scl_s[:, 5:6]
                # Only e4 reads the gram — its PSUM slot frees immediately,
                # so later grams never stall PE on the pool semaphore.  The
                # e1/e0 heads are ACT Squares chained off es2/es1 (their only
                # consumers are WLAG-slack weighted matmuls + each other).
                nc.scalar.activation(
                    es[4][:, 0:w], gt[:, 0:w], Exp, bias=zb, scale=scl_s[:, 0:1],
                )
                nc.vector.tensor_mul(es[3][:, 0:w], es[4][:, 0:w], es[4][:, 0:w])
                nc.vector.tensor_mul(es[2][:, 0:w], es[3][:, 0:w], es[3][:, 0:w])
                nc.scalar.activation(
                    es[1][:, 0:a], es[2][:, 0:a], Square, bias=zb, scale=1.0,
                )
                nc.scalar.activation(
                    es[0][:, 0:a], es[1][:, 0:a], Square, bias=zb, scale=1.0,
                )
                if a < w:
                    nc.vector.tensor_mul(es[1][:, a:w], es[2][:, a:w], es[2][:, a:w])
                    if bi >= NB - NPOOL_FREE:
                        nc.vector.tensor_mul(es[0][:, a:w], es[1][:, a:w], es[1][:, a:w])
                    else:
                        nc.gpsimd.tensor_mul(es[0][:, a:w], es[1][:, a:w], es[1][:, a:w])
                pending.append((bi, es))
            for item in pending:
                emit_main(*item)
                pending2.append(item)
            for k, item in enumerate(pending2):
                emit_q0_tail(*item, stop=(k == len(pending2) - 1))

            # tail: hi drains on DVE while the low-q matmuls still run; lo on
            # the by-then-idle ACT; one bf16 DMA.
            stg = singles.tile([128, NQ * NI * C], fp32)
            nc.vector.tensor_copy(stg[:, 2 * NI * C :], rq_hi)
            nc.scalar.activation(stg[:, 0 : 2 * NI * C], rq_lo, Copy)
            nc.sync.dma_start(out=rout[:], in_=stg)

    nc.compile()
    return nc


def _prep(source, target, source_label, target_logits):
    X = np.concatenate([np.asarray(source), np.asarray(target)], axis=0)
    X64 = X.astype(np.float64)
    sq = np.einsum("nd,nd->n", X64, X64)
    colsum = X64.sum(axis=0)
    sum_l2 = 2.0 * N * sq.sum() - 2.0 * (colsum @ colsum)
    bw = sum_l2 / (N * N - N) / (2.0 ** (NQ // 2))
    cq = np.array([1.0 / (bw * 2.0**q) for q in range(NQ)])  # [5]

    sl = np.asarray(source_label, np.float64)
    tl = np.asarray(target_logits, np.float64)
    ssum = sl.sum(0)
    s_norm = np.where(ssum > 0, sl / np.where(ssum > 0, ssum, 1.0), 0.0)
    tsum = tl.sum(0)
    t_norm = np.where(tsum > 0, tl / np.where(tsum > 0, tsum, 1.0), 0.0)
    s_pres = np.zeros(C)
    np.add.at(s_pres, sl.argmax(1), 1.0)
    t_pres = np.zeros(C)
    np.add.at(t_pres, tl.argmax(1), 1.0)
    common = ((s_pres > 0) & (t_pres > 0)).astype(np.float64)
    V = np.concatenate([s_norm * common, -t_norm * common], axis=0)  # [N, C]

    fq = np.exp(-np.outer(cq, sq))                        # [5, N]

    X8 = X.astype(F8NP)                                   # [N, D]
    xt8 = np.ascontiguousarray(
        X8.T.reshape(NKC, 128, N).transpose(1, 0, 2)      # [128, 8, N]
    )
    return X, sq, cq, V, fq, xt8


def _core_inputs(c, cq, V, fq, xt8):
    gtiles = [(4 * c + s) % 32 for s in S0]
    xtb = np.ascontiguousarray(
        xt8.reshape(128, NKC, NT, 128)[:, :, gtiles, :].reshape(128, NKC, M * 128)
    )
    # vt2[q, t] = 2 V f_q at global tile gtiles[t]
    Vt = V.reshape(NT, 128, C)[gtiles]                    # [M, 128, C]
    fqt = fq.reshape(NQ, NT, 128)[:, gtiles]              # [NQ, M, 128]
    Vq = Vt[None] * fqt[..., None]                        # [NQ, M, 128, C]
    vt2 = (2.0 * Vq).transpose(2, 0, 1, 3).reshape(128, NQ * M * C)
    # vt1 slots: 0..3 = d16 jobs (j = position 8+... tile 16+x), 4..7 = diag x
    vt1 = np.zeros((128, NQ, 8, C))
    for x in range(4):
        jpos = S0.index(x + 16)
        vt1[:, :, x, :] = Vq[:, jpos].transpose(1, 0, 2)
        vt1[:, :, 4 + x, :] = Vq[:, S0.index(x)].transpose(1, 0, 2)
    vt1 = vt1.reshape(128, NQ * 8 * C)
    scl = np.zeros((128, 8))
    for k in range(NQ):
        scl[:, k] = 2.0 * cq[4 - k]
    btab = np.ascontiguousarray(
        np.concatenate([vt2, vt1, scl], axis=1)
    ).astype(BFNP)
    return {"xtb": xtb, "btab": btab}


def _postprocess(results, sq, cq, V):
    # loss = 1/12 sum_q sum_i alpha_q[i] * (sum_cls V[i,cls] R_q[i,cls])
    loss = 0.0
    for c in range(NCORES):
        r = np.asarray(results[c]["r_out"], np.float64).reshape(128, NQ, NI, C)
        for s in range(NI):
            gt_ = (4 * c + S0[s]) % 32
            gi = gt_ * 128 + np.arange(128)
            alpha = np.exp(-np.outer(cq, sq[gi]))         # [NQ, 128]
            loss += np.einsum("qp,pc,pqc->", alpha, V[gi], r[:, :, s, :])
    return loss / C


def _run(in_maps, trace=False, **kw):
    global _BUILT
    if _BUILT is None:
        _BUILT = _build_program()
    return run_bass_kernel_spmd(_BUILT, in_maps, list(range(NCORES)), trace=trace, **kw)


def kernel(source, target, source_label, target_logits, _trace=False, _ret_bkr=False):
    X, sq, cq, V, fq, xt8 = _prep(source, target, source_label, target_logits)
    in_maps = [_core_inputs(c, cq, V, fq, xt8) for c in range(NCORES)]
    bkr = None
    for attempt in range(3):
        try:
            bkr = _run(in_maps, trace=_trace)
            break
        except Exception:
            if attempt == 2:
                raise
            import time as _time

            _time.sleep(2.0)
    loss = _postprocess(bkr.results, sq, cq, V)
    out = np.float32(loss)
    if _ret_bkr:
        return out, bkr
    return out
